# revision 25
# baseline (speedup 1.0000x reference)
"""Multi-head causal attention (B=4, T=2048, C=1024, H=16, DH=64) on 8
Trainium2 NeuronCores.

Sharding: data-parallel over batch (4) x tensor-parallel over head groups
(2 groups of 8 heads). Core c handles batch c//2... (b = c//2, g = c%2).
Each core computes its 8 heads end-to-end (QKV projections, causal
attention, partial output projection); the host sums the two head-group
partials per batch and adds the bias (the TP unshard/all-reduce).

Kernel math (per core, all matmuls in float32r at full PE rate):
  xT [C, T] (host-pretransposed) ->
  qT/kT [2*DH, T] per head pair (heads packed on partitions),
  v [T, DH] per head, padded with 32 ones-columns so the attention-value
  matmul also produces the softmax row-sums (rows 64:96 of the output).
  Scores are computed transposed: ST[k, q] = k @ qT, exp on ScalarE
  (no max subtraction needed: |scores| <= ~6 in fp32), causal tri-mask
  multiply on diagonal 128-blocks, AV accumulated over k-tiles in PSUM,
  normalize by reciprocal of the ones-column row, then y += outT.T @ WoT.
"""

import numpy as np

import concourse.bass as bass
import concourse.tile as tile
from concourse import bacc, mybir
from concourse.bass_utils import run_bass_kernel_spmd

P = 128
DH = 64
ONES_COLS = 32  # v augmented to DH + ONES_COLS columns; rows 64:96 = rowsum
F32 = mybir.dt.float32
F32R = mybir.dt.float32r
BF16 = mybir.dt.bfloat16
EXP = mybir.ActivationFunctionType.Exp
MULT = mybir.AluOpType.mult


def build_nc(T=2048, C=1024, HL=8, n_devices=8):
    """Build the per-core SPMD kernel. HL = heads per core."""
    NPAIR = HL // 2
    CO = C // P          # contraction tiles over C
    TQ = min(512, T)     # query-chunk (free dim of score tiles)
    NTQ = T // TQ
    NKT = T // P         # key tiles
    KPQ = TQ // P        # key tiles per query chunk
    HDH = HL * DH
    VW = DH + ONES_COLS  # 96
    CW = min(512, C)     # output-projection column chunk
    NCC = C // CW
    MQ = TQ // P         # 128-row chunks per query chunk (output proj M)

    nc = bacc.Bacc("TRN2", target_bir_lowering=False, debug=False,
                   enable_asserts=True, num_devices=n_devices)

    xT_d = nc.dram_tensor("xT", [C, T], F32R, kind="ExternalInput").ap()
    wq_d = nc.dram_tensor("wq", [NPAIR, CO, P, P], F32R, kind="ExternalInput").ap()
    wk_d = nc.dram_tensor("wk", [NPAIR, CO, P, P], F32R, kind="ExternalInput").ap()
    wv_d = nc.dram_tensor("wv", [CO, P, HDH], F32R, kind="ExternalInput").ap()
    wo_d = nc.dram_tensor("wo", [NPAIR, P, C], F32R, kind="ExternalInput").ap()
    ident_d = nc.dram_tensor("ident", [P, P], BF16, kind="ExternalInput").ap()
    trineg_d = nc.dram_tensor("trineg", [P, P], BF16, kind="ExternalInput").ap()
    ones_d = nc.dram_tensor("ones", [P, NKT, HL, ONES_COLS], F32R,
                            kind="ExternalInput").ap()
    y_d = nc.dram_tensor("y", [T, C], F32, kind="ExternalOutput").ap()

    xT_v = xT_d.rearrange("(co ci) t -> ci co t", ci=P)

    with tile.TileContext(nc) as tc:
        with tc.tile_pool(name="persist", bufs=1) as persist:
            # q/k, pair-packed on partitions: [p<64]=even head, [p>=64]=odd
            qk_sb = persist.tile([P, 2, NPAIR, T], F32R)
            # v, t-tiled on partitions, per head DH cols + ones columns
            v_sb = persist.tile([P, NKT, HL, VW], F32R)
            nc.scalar.dma_start(v_sb[:, :, :, DH:VW], ones_d[:])

            # ---------------- phase A: projections ----------------
            with tc.tile_pool(name="xq", bufs=1) as xqp, \
                 tc.tile_pool(name="wsb", bufs=1) as wsb, \
                 tc.tile_pool(name="psA", bufs=8, space="PSUM") as psA:
                wq_sb = wsb.tile([P, NPAIR, CO, P], F32R)
                wk_sb = wsb.tile([P, NPAIR, CO, P], F32R)
                wv_sb = wsb.tile([P, CO, HDH], F32R)
                xqs = []
                for q in range(NTQ):
                    xq = xqp.tile([P, CO, TQ], F32R, name=f"xq{q % 2}")
                    nc.sync.dma_start(xq[:], xT_v[:, :, q * TQ:(q + 1) * TQ])
                    xqs.append(xq)
                    if q == 0:
                        nc.gpsimd.dma_start(
                            wq_sb[:], wq_d.rearrange("q co ci m -> ci q co m"))
                        nc.gpsimd.dma_start(
                            wk_sb[:], wk_d.rearrange("q co ci m -> ci q co m"))
                        nc.gpsimd.dma_start(
                            wv_sb[:], wv_d.rearrange("co ci n -> ci co n"))

                for q in range(NTQ):
                    xq = xqs[q]
                    # q/k projections for this query chunk
                    for qk in range(2):
                        w_sb = wq_sb if qk == 0 else wk_sb
                        for pair in range(NPAIR):
                            ps = psA.tile([P, TQ], F32)
                            for co in range(CO):
                                nc.tensor.matmul(ps[:], w_sb[:, pair, co],
                                                 xq[:, co, :],
                                                 start=(co == 0),
                                                 stop=(co == CO - 1))
                            nc.vector.tensor_copy(
                                qk_sb[:, qk, pair, q * TQ:(q + 1) * TQ], ps[:])
                    # v projection for this chunk's key tiles
                    for tkr in range(KPQ):
                        tk = q * KPQ + tkr
                        ps = psA.tile([P, HDH], F32)
                        for co in range(CO):
                            nc.tensor.matmul(ps[:], xq[:, co, tkr * P:(tkr + 1) * P],
                                             wv_sb[:, co],
                                             start=(co == 0), stop=(co == CO - 1))
                        nc.vector.tensor_copy(
                            v_sb[:, tk, :, 0:DH],
                            ps.rearrange("p (h d) -> p h d", d=DH))

            # ---------------- phase B: attention + out-projection ----------------
            with tc.tile_pool(name="wo2", bufs=1) as wo2, \
                 tc.tile_pool(name="ptp", bufs=8) as ptp, \
                 tc.tile_pool(name="otsb", bufs=2) as otsb, \
                 tc.tile_pool(name="zp", bufs=4) as zp, \
                 tc.tile_pool(name="ysb", bufs=3) as ysbp, \
                 tc.tile_pool(name="stp", bufs=3, space="PSUM") as stp, \
                 tc.tile_pool(name="otp", bufs=1, space="PSUM") as otp:
                wo_sb = wo2.tile([P, NPAIR, C], F32R)
                nc.scalar.dma_start(wo_sb[:], wo_d.rearrange("q p c -> p q c"))
                ident_sb = wo2.tile([P, P], BF16)
                nc.scalar.dma_start(ident_sb[:], ident_d[:])
                trineg_sb = wo2.tile([P, P], BF16)
                nc.scalar.dma_start(trineg_sb[:], trineg_d[:])

                LAG = 3  # AV lags score/exp by LAG (pair,tk) units
                pending = []
                for tq in range(NTQ):
                    outT = otsb.tile([P, NPAIR, TQ], F32R)
                    ntk = KPQ * tq + KPQ
                    units = [(pair, tk)
                             for pair in range(NPAIR)
                             for tk in range(ntk)]
                    pts = {}
                    ots_by_pair = {}

                    def emit_score_exp(j):
                        pair, tk = units[j]
                        rel = tk - KPQ * tq
                        cs = max(rel, 0) * P
                        n = TQ - cs
                        st = stp.tile([P, 2, TQ], F32, name="st")
                        for hh in range(2):
                            pb = hh * DH
                            if rel >= 0:
                                # causal mask: accumulate -1e9 upper triangle
                                # into the first 128 columns
                                nc.tensor.matmul(
                                    st[:, hh, :P], ident_sb[:], trineg_sb[:],
                                    start=True, stop=False,
                                    skip_group_check=True)
                            nc.tensor.matmul(
                                st[:, hh, :n],
                                qk_sb[pb:pb + DH, 1, pair, tk * P:(tk + 1) * P],
                                qk_sb[pb:pb + DH, 0, pair,
                                      tq * TQ + cs:(tq + 1) * TQ],
                                start=(rel < 0), stop=True,
                                skip_group_check=True)
                        pt = ptp.tile([P, 2, TQ], F32R, name="pt")
                        nc.scalar.activation(pt[:, :, :n], st[:, :, :n], EXP,
                                             scale=DH ** -0.5)
                        pts[j] = (pt, cs, n)

                    def emit_av(j):
                        pair, tk = units[j]
                        if pair not in ots_by_pair:
                            ots_by_pair[pair] = [
                                otp.tile([P, TQ], F32, name=f"ot{h}")
                                for h in range(2)]
                        ots = ots_by_pair[pair]
                        pt, cs, n = pts.pop(j)
                        for hh in range(2):
                            nc.tensor.matmul(
                                ots[hh][0:VW, cs:TQ],
                                v_sb[:, tk, pair * 2 + hh, :],
                                pt[:, hh, :n],
                                start=(tk == 0), stop=(tk == ntk - 1))
                        if tk == ntk - 1:
                            # normalize: rows 64:96 of ots = softmax row-sums
                            for hh in range(2):
                                zcp = zp.tile([P, TQ], F32, name="zcp")
                                zinv = zp.tile([P, TQ], F32, name="zinv")
                                nc.vector.tensor_copy(
                                    zcp[DH:DH + ONES_COLS, :],
                                    ots[hh][DH:DH + ONES_COLS, :])
                                # custom-DVE op needs full-partition operands;
                                # rows outside 64:96 are junk, never read
                                nc.vector.reciprocal_approx_fast(zinv[:], zcp[:])
                                for half in range(DH // ONES_COLS):
                                    hs = half * ONES_COLS
                                    nc.vector.tensor_tensor(
                                        outT[hh * DH + hs:
                                             hh * DH + hs + ONES_COLS, pair, :],
                                        ots[hh][hs:hs + ONES_COLS, :],
                                        zinv[DH:DH + ONES_COLS, :], MULT)
                            del ots_by_pair[pair]

                    def mk_outproj(tq_, outT_, m, cc):
                        # two half-group closures (finer weave granularity);
                        # they share one accumulating PSUM tile
                        box = {}

                        def emit_lo():
                            box["ps"] = stp.tile([P, CW], F32, name="st")
                            for pair in range(NPAIR // 2):
                                nc.tensor.matmul(
                                    box["ps"][:],
                                    outT_[:, pair, m * P:(m + 1) * P],
                                    wo_sb[:, pair, cc * CW:(cc + 1) * CW],
                                    start=(pair == 0), stop=False)

                        def emit_hi():
                            t0 = tq_ * TQ + m * P
                            for pair in range(NPAIR // 2, NPAIR):
                                nc.tensor.matmul(
                                    box["ps"][:],
                                    outT_[:, pair, m * P:(m + 1) * P],
                                    wo_sb[:, pair, cc * CW:(cc + 1) * CW],
                                    start=False, stop=(pair == NPAIR - 1))
                            yt = ysbp.tile([P, CW], F32)
                            nc.vector.tensor_copy(yt[:], box["ps"][:])
                            nc.sync.dma_start(
                                y_d[t0:t0 + P, cc * CW:(cc + 1) * CW], yt[:])
                        return [emit_lo, emit_hi]

                    # weave the PREVIOUS tq's output projection into this
                    # tq's ACT-paced unit stream as dense PE filler
                    stride = max(1, (len(units) + LAG) // max(len(pending), 1))
                    for j in range(len(units) + LAG):
                        if j < len(units):
                            emit_score_exp(j)
                        if j >= LAG:
                            emit_av(j - LAG)
                        if pending and j % stride == stride - 1:
                            pending.pop(0)()
                    while pending:
                        pending.pop(0)()
                    pending = [half
                               for m in range(MQ) for cc in range(NCC)
                               for half in mk_outproj(tq, outT, m, cc)]
                # final tq's output projection
                while pending:
                    pending.pop(0)()

    nc.compile()
    return nc


def pack_inputs(x_b, Wq_g, Wk_g, Wv_g, Wo_g):
    """Per-core input map. x_b [T, C]; W{q,k,v}_g [HL, C, DH]; Wo_g [C, HL*DH]."""
    T = x_b.shape[0]
    HL, C, _ = Wq_g.shape
    NPAIR = HL // 2
    CO = C // P

    def pack_qk(w):
        # [pair, co, ci, hh*DH+d] = w[2*pair+hh, co*P+ci, d]
        return np.ascontiguousarray(
            w.reshape(NPAIR, 2, CO, P, DH).transpose(0, 2, 3, 1, 4)
            .reshape(NPAIR, CO, P, P))

    wv = np.ascontiguousarray(
        Wv_g.reshape(HL, CO, P, DH).transpose(1, 2, 0, 3).reshape(CO, P, HL * DH))
    wo = np.ascontiguousarray(Wo_g.T).reshape(NPAIR, P, C)
    import ml_dtypes
    ident = np.eye(P, dtype=ml_dtypes.bfloat16)
    trineg = np.where(np.arange(P)[None, :] < np.arange(P)[:, None],
                      -1e9, 0.0).astype(ml_dtypes.bfloat16)
    return {
        "xT": np.ascontiguousarray(x_b.T),
        "wq": pack_qk(Wq_g),
        "wk": pack_qk(Wk_g),
        "wv": wv,
        "wo": wo,
        "ident": ident,
        "trineg": trineg,
        "ones": np.ones((P, T // P, HL, ONES_COLS), dtype=np.float32),
    }


_NC_CACHE = {}


def kernel(x, Wq, Wk, Wv, Wo, bo):
    x = np.asarray(x, dtype=np.float32)
    Wq = np.asarray(Wq, dtype=np.float32)
    Wk = np.asarray(Wk, dtype=np.float32)
    Wv = np.asarray(Wv, dtype=np.float32)
    Wo = np.asarray(Wo, dtype=np.float32)
    bo = np.asarray(bo, dtype=np.float32)

    B, T, C = x.shape
    H = Wq.shape[0]
    HL = H // 2  # 2 head groups

    key = (T, C, HL)
    if key not in _NC_CACHE:
        _NC_CACHE[key] = build_nc(T=T, C=C, HL=HL)
    nc = _NC_CACHE[key]

    in_maps = []
    for core in range(8):
        b, g = core // 2, core % 2
        hs = slice(g * HL, (g + 1) * HL)
        in_maps.append(pack_inputs(
            x[b], Wq[hs], Wk[hs], Wv[hs],
            Wo[:, g * HL * DH:(g + 1) * HL * DH]))

    res = run_bass_kernel_spmd(nc, in_maps, core_ids=list(range(8)))
    y = np.stack([res.results[2 * b]["y"] + res.results[2 * b + 1]["y"] + bo
                  for b in range(B)])
    return y.astype(np.float32)


# revision 27
# speedup vs baseline: 1.0149x; 1.0149x over previous
"""Multi-head causal attention (B=4, T=2048, C=1024, H=16, DH=64) on 8
Trainium2 NeuronCores.

Sharding: data-parallel over batch (4) x tensor-parallel over head groups
(2 groups of 8 heads). Core c handles batch c//2... (b = c//2, g = c%2).
Each core computes its 8 heads end-to-end (QKV projections, causal
attention, partial output projection); the host sums the two head-group
partials per batch and adds the bias (the TP unshard/all-reduce).

Kernel math (per core, all matmuls in float32r at full PE rate):
  xT [C, T] (host-pretransposed) ->
  qT/kT [2*DH, T] per head pair (heads packed on partitions),
  v [T, DH] per head, padded with 32 ones-columns so the attention-value
  matmul also produces the softmax row-sums (rows 64:96 of the output).
  Scores are computed transposed: ST[k, q] = k @ qT, exp on ScalarE
  (no max subtraction needed: |scores| <= ~6 in fp32), causal tri-mask
  multiply on diagonal 128-blocks, AV accumulated over k-tiles in PSUM,
  normalize by reciprocal of the ones-column row, then y += outT.T @ WoT.
"""

import numpy as np

import concourse.bass as bass
import concourse.tile as tile
from concourse import bacc, mybir
from concourse.bass_utils import run_bass_kernel_spmd

P = 128
DH = 64
ONES_COLS = 32  # v augmented to DH + ONES_COLS columns; rows 64:96 = rowsum
F32 = mybir.dt.float32
F32R = mybir.dt.float32r
BF16 = mybir.dt.bfloat16
EXP = mybir.ActivationFunctionType.Exp
MULT = mybir.AluOpType.mult


def build_nc(T=2048, C=1024, HL=8, n_devices=8):
    """Build the per-core SPMD kernel. HL = heads per core."""
    NPAIR = HL // 2
    CO = C // P          # contraction tiles over C
    TQ = min(512, T)     # query-chunk (free dim of score tiles)
    NTQ = T // TQ
    NKT = T // P         # key tiles
    KPQ = TQ // P        # key tiles per query chunk
    HDH = HL * DH
    VW = DH + ONES_COLS  # 96
    CW = min(512, C)     # output-projection column chunk
    NCC = C // CW
    MQ = TQ // P         # 128-row chunks per query chunk (output proj M)

    nc = bacc.Bacc("TRN2", target_bir_lowering=False, debug=False,
                   enable_asserts=True, num_devices=n_devices)

    xT_d = nc.dram_tensor("xT", [C, T], F32R, kind="ExternalInput").ap()
    wq_d = nc.dram_tensor("wq", [NPAIR, CO, P, P], F32R, kind="ExternalInput").ap()
    wk_d = nc.dram_tensor("wk", [NPAIR, CO, P, P], F32R, kind="ExternalInput").ap()
    wv_d = nc.dram_tensor("wv", [CO, P, HDH], F32R, kind="ExternalInput").ap()
    wo_d = nc.dram_tensor("wo", [NPAIR, P, C], F32R, kind="ExternalInput").ap()
    ident_d = nc.dram_tensor("ident", [P, P], BF16, kind="ExternalInput").ap()
    trineg_d = nc.dram_tensor("trineg", [P, P], BF16, kind="ExternalInput").ap()
    ones_d = nc.dram_tensor("ones", [P, NKT, HL, ONES_COLS], F32R,
                            kind="ExternalInput").ap()
    y_d = nc.dram_tensor("y", [T, C], F32, kind="ExternalOutput").ap()

    xT_v = xT_d.rearrange("(co ci) t -> ci co t", ci=P)

    with tile.TileContext(nc) as tc:
        with tc.tile_pool(name="persist", bufs=1) as persist:
            # q/k, pair-packed on partitions: [p<64]=even head, [p>=64]=odd
            qk_sb = persist.tile([P, 2, NPAIR, T], F32R)
            # v, t-tiled on partitions, per head DH cols + ones columns
            v_sb = persist.tile([P, NKT, HL, VW], F32R)
            nc.scalar.dma_start(v_sb[:, :, :, DH:VW], ones_d[:])

            # ---------------- phase A: projections ----------------
            with tc.tile_pool(name="xq", bufs=1) as xqp, \
                 tc.tile_pool(name="wsb", bufs=1) as wsb, \
                 tc.tile_pool(name="psA", bufs=8, space="PSUM") as psA:
                wq_sb = wsb.tile([P, NPAIR, CO, P], F32R)
                wk_sb = wsb.tile([P, NPAIR, CO, P], F32R)
                wv_sb = wsb.tile([P, CO, HDH], F32R)
                xqs = []
                for q in range(NTQ):
                    xq = xqp.tile([P, CO, TQ], F32R, name=f"xq{q % 2}")
                    nc.sync.dma_start(xq[:], xT_v[:, :, q * TQ:(q + 1) * TQ])
                    xqs.append(xq)
                    if q == 0:
                        nc.gpsimd.dma_start(
                            wq_sb[:], wq_d.rearrange("q co ci m -> ci q co m"))
                        nc.gpsimd.dma_start(
                            wk_sb[:], wk_d.rearrange("q co ci m -> ci q co m"))
                        nc.gpsimd.dma_start(
                            wv_sb[:], wv_d.rearrange("co ci n -> ci co n"))

                for q in range(NTQ):
                    xq = xqs[q]
                    # q/k projections for this query chunk
                    for qk in range(2):
                        w_sb = wq_sb if qk == 0 else wk_sb
                        for pair in range(NPAIR):
                            ps = psA.tile([P, TQ], F32)
                            for co in range(CO):
                                nc.tensor.matmul(ps[:], w_sb[:, pair, co],
                                                 xq[:, co, :],
                                                 start=(co == 0),
                                                 stop=(co == CO - 1))
                            nc.vector.tensor_copy(
                                qk_sb[:, qk, pair, q * TQ:(q + 1) * TQ], ps[:])
                    # v projection for this chunk's key tiles
                    for tkr in range(KPQ):
                        tk = q * KPQ + tkr
                        ps = psA.tile([P, HDH], F32)
                        for co in range(CO):
                            nc.tensor.matmul(ps[:], xq[:, co, tkr * P:(tkr + 1) * P],
                                             wv_sb[:, co],
                                             start=(co == 0), stop=(co == CO - 1))
                        nc.vector.tensor_copy(
                            v_sb[:, tk, :, 0:DH],
                            ps.rearrange("p (h d) -> p h d", d=DH))

            # ---------------- phase B: attention + out-projection ----------------
            with tc.tile_pool(name="wo2", bufs=1) as wo2, \
                 tc.tile_pool(name="ptp", bufs=8) as ptp, \
                 tc.tile_pool(name="otsb", bufs=2) as otsb, \
                 tc.tile_pool(name="zp", bufs=4) as zp, \
                 tc.tile_pool(name="ysb", bufs=3) as ysbp, \
                 tc.tile_pool(name="stp", bufs=3, space="PSUM") as stp, \
                 tc.tile_pool(name="otp", bufs=1, space="PSUM") as otp:
                wo_sb = wo2.tile([P, NPAIR, C], F32R)
                nc.scalar.dma_start(wo_sb[:], wo_d.rearrange("q p c -> p q c"))
                ident_sb = wo2.tile([P, P], BF16)
                nc.scalar.dma_start(ident_sb[:], ident_d[:])
                trineg_sb = wo2.tile([P, P], BF16)
                nc.scalar.dma_start(trineg_sb[:], trineg_d[:])

                LAG = 4  # AV lags score/exp by LAG (pair,tk) units
                pending = []
                for tq in range(NTQ):
                    outT = otsb.tile([P, NPAIR, TQ], F32R)
                    ntk = KPQ * tq + KPQ
                    units = [(pair, tk)
                             for pair in range(NPAIR)
                             for tk in range(ntk)]
                    pts = {}
                    ots_by_pair = {}

                    def emit_score_exp(j):
                        pair, tk = units[j]
                        rel = tk - KPQ * tq
                        cs = max(rel, 0) * P
                        n = TQ - cs
                        st = stp.tile([P, 2, TQ], F32, name="st")
                        for hh in range(2):
                            pb = hh * DH
                            if rel >= 0:
                                # causal mask: accumulate -1e9 upper triangle
                                # into the first 128 columns
                                nc.tensor.matmul(
                                    st[:, hh, :P], ident_sb[:], trineg_sb[:],
                                    start=True, stop=False,
                                    skip_group_check=True)
                            nc.tensor.matmul(
                                st[:, hh, :n],
                                qk_sb[pb:pb + DH, 1, pair, tk * P:(tk + 1) * P],
                                qk_sb[pb:pb + DH, 0, pair,
                                      tq * TQ + cs:(tq + 1) * TQ],
                                start=(rel < 0), stop=True,
                                skip_group_check=True)
                        pt = ptp.tile([P, 2, TQ], F32R, name="pt")
                        nc.scalar.activation(pt[:, :, :n], st[:, :, :n], EXP,
                                             scale=DH ** -0.5)
                        pts[j] = (pt, cs, n)

                    def emit_av(j):
                        pair, tk = units[j]
                        if pair not in ots_by_pair:
                            ots_by_pair[pair] = [
                                otp.tile([P, TQ], F32, name=f"ot{h}")
                                for h in range(2)]
                        ots = ots_by_pair[pair]
                        pt, cs, n = pts.pop(j)
                        for hh in range(2):
                            nc.tensor.matmul(
                                ots[hh][0:VW, cs:TQ],
                                v_sb[:, tk, pair * 2 + hh, :],
                                pt[:, hh, :n],
                                start=(tk == 0), stop=(tk == ntk - 1))
                        if tk == ntk - 1:
                            # normalize: rows 64:96 of ots = softmax row-sums
                            for hh in range(2):
                                zcp = zp.tile([P, TQ], F32, name="zcp")
                                zinv = zp.tile([P, TQ], F32, name="zinv")
                                nc.vector.tensor_copy(
                                    zcp[DH:DH + ONES_COLS, :],
                                    ots[hh][DH:DH + ONES_COLS, :])
                                # custom-DVE op needs full-partition operands;
                                # rows outside 64:96 are junk, never read
                                nc.vector.reciprocal_approx_fast(zinv[:], zcp[:])
                                for half in range(DH // ONES_COLS):
                                    hs = half * ONES_COLS
                                    nc.vector.tensor_tensor(
                                        outT[hh * DH + hs:
                                             hh * DH + hs + ONES_COLS, pair, :],
                                        ots[hh][hs:hs + ONES_COLS, :],
                                        zinv[DH:DH + ONES_COLS, :], MULT)
                            del ots_by_pair[pair]

                    def mk_outproj(tq_, outT_, m, cc):
                        def emit():
                            t0 = tq_ * TQ + m * P
                            ps = stp.tile([P, CW], F32, name="st")
                            for pair in range(NPAIR):
                                nc.tensor.matmul(
                                    ps[:],
                                    outT_[:, pair, m * P:(m + 1) * P],
                                    wo_sb[:, pair, cc * CW:(cc + 1) * CW],
                                    start=(pair == 0), stop=(pair == NPAIR - 1))
                            yt = ysbp.tile([P, CW], F32)
                            nc.vector.tensor_copy(yt[:], ps[:])
                            nc.sync.dma_start(
                                y_d[t0:t0 + P, cc * CW:(cc + 1) * CW], yt[:])
                        return emit

                    # weave the PREVIOUS tq's output projection into this
                    # tq's ACT-paced unit stream as dense PE filler
                    stride = max(1, (len(units) + LAG) // max(len(pending), 1))
                    for j in range(len(units) + LAG):
                        if j < len(units):
                            emit_score_exp(j)
                        if j >= LAG:
                            emit_av(j - LAG)
                        if pending and j % stride == stride - 1:
                            pending.pop(0)()
                    while pending:
                        pending.pop(0)()
                    pending = [mk_outproj(tq, outT, m, cc)
                               for m in range(MQ) for cc in range(NCC)]
                # final tq's output projection
                while pending:
                    pending.pop(0)()

    nc.compile()
    return nc


def pack_inputs(x_b, Wq_g, Wk_g, Wv_g, Wo_g):
    """Per-core input map. x_b [T, C]; W{q,k,v}_g [HL, C, DH]; Wo_g [C, HL*DH]."""
    T = x_b.shape[0]
    HL, C, _ = Wq_g.shape
    NPAIR = HL // 2
    CO = C // P

    def pack_qk(w):
        # [pair, co, ci, hh*DH+d] = w[2*pair+hh, co*P+ci, d]
        return np.ascontiguousarray(
            w.reshape(NPAIR, 2, CO, P, DH).transpose(0, 2, 3, 1, 4)
            .reshape(NPAIR, CO, P, P))

    wv = np.ascontiguousarray(
        Wv_g.reshape(HL, CO, P, DH).transpose(1, 2, 0, 3).reshape(CO, P, HL * DH))
    wo = np.ascontiguousarray(Wo_g.T).reshape(NPAIR, P, C)
    import ml_dtypes
    ident = np.eye(P, dtype=ml_dtypes.bfloat16)
    trineg = np.where(np.arange(P)[None, :] < np.arange(P)[:, None],
                      -1e9, 0.0).astype(ml_dtypes.bfloat16)
    return {
        "xT": np.ascontiguousarray(x_b.T),
        "wq": pack_qk(Wq_g),
        "wk": pack_qk(Wk_g),
        "wv": wv,
        "wo": wo,
        "ident": ident,
        "trineg": trineg,
        "ones": np.ones((P, T // P, HL, ONES_COLS), dtype=np.float32),
    }


_NC_CACHE = {}


def kernel(x, Wq, Wk, Wv, Wo, bo):
    x = np.asarray(x, dtype=np.float32)
    Wq = np.asarray(Wq, dtype=np.float32)
    Wk = np.asarray(Wk, dtype=np.float32)
    Wv = np.asarray(Wv, dtype=np.float32)
    Wo = np.asarray(Wo, dtype=np.float32)
    bo = np.asarray(bo, dtype=np.float32)

    B, T, C = x.shape
    H = Wq.shape[0]
    HL = H // 2  # 2 head groups

    key = (T, C, HL)
    if key not in _NC_CACHE:
        _NC_CACHE[key] = build_nc(T=T, C=C, HL=HL)
    nc = _NC_CACHE[key]

    in_maps = []
    for core in range(8):
        b, g = core // 2, core % 2
        hs = slice(g * HL, (g + 1) * HL)
        in_maps.append(pack_inputs(
            x[b], Wq[hs], Wk[hs], Wv[hs],
            Wo[:, g * HL * DH:(g + 1) * HL * DH]))

    res = run_bass_kernel_spmd(nc, in_maps, core_ids=list(range(8)))
    y = np.stack([res.results[2 * b]["y"] + res.results[2 * b + 1]["y"] + bo
                  for b in range(B)])
    return y.astype(np.float32)
